# revision 5
# baseline (speedup 1.0000x reference)
"""Trainium2 Bass kernel for BioNormalizedPolynomialCKN1D.

Computes, for x[B=64, L=4096, CIN=64], k[7, 64, 128], b[128], g, c (scalars):
    dot = conv_valid(x, k); ws = conv_valid(x*x, ones)       # [B, 4090, *]
    out = (g * dot / sqrt(ws + eps))**2 + b

Strategy (8 NeuronCores, data-parallel over batch, 8 batches/core):
  - Host packs x even/odd interleaved + channel-transposed:
      x_eo[b, p, ci, m] = x[b, 2m+p, ci]  -> SBUF tile XEO[128, M+PAD]
    with partitions = (parity*64 + ci). The 7-tap conv becomes 4
    accumulating K<=128 fp32r matmuls per output parity (tap pairs sit on
    the two 64-partition decks at one column offset) — no on-chip
    transposes or shifts.
  - Windowed sum-of-squares via a two-level scheme:
      s1[p, m] = sum_ci x^2  (one K=128 matmul, M=2)
      s1m[8, C]: row-pairs = s1 at col offsets 0..3 (4 tiny SBUF DMAs)
      ws broadcast [128, N] = ones.T @ s1m   (K=8 matmul)
    where the SAME s1m serves both parities: even taps = rows 0..6,
    odd taps = rows 1..7 (selected by zero rows in the ones lhsT).
  - Pointwise, spread across DVE/ACT (GPSIMD avoided entirely - it is
    ~17x slower than spec on tensor_scalar and its SBUF port is shared
    with DVE): r2 = reciprocal_approx_fast(ws) [DVE], dsq = Square(g*dot)
    [ACT], t = dsq*r2 [DVE], out = t + b [alternating ACT bias-add / DVE
    tensor_scalar for balance].
  - Device output layout [b, parity, F, m]; host inverse-permutes.

eps: ws ~ chi2(448) >= O(100) for this input distribution, so eps=1e-7
is relatively < 1e-9 and the fast path omits it. The general path (c!=0)
applies it exactly via the ACT Sqrt bias.
"""

import numpy as np
from contextlib import ExitStack

import concourse.tile as tile
from concourse import mybir, bacc
from concourse.bass_utils import run_bass_kernel_spmd

_B, _L, _CIN, _F, _KS = 64, 4096, 64, 128, 7
_LP = _L - _KS + 1           # 4090
_M = _L // 2                 # 2048 columns per parity
_PAD = 8
_MT = 512                    # matmul moving tile (one PSUM bank of fp32)
_NCORES = 8
_BPC = _B // _NCORES
_EPS = 1e-7

F32 = mybir.dt.float32
F32R = mybir.dt.float32r
BF16 = mybir.dt.bfloat16

_prog_cache = {}


def _build_program(g_s: float, c_s: float):
    nc = bacc.Bacc("TRN2", target_bir_lowering=False)
    x_in = nc.dram_tensor("x", [_BPC, 2, _CIN, _M + _PAD], F32, kind="ExternalInput")
    kw_in = nc.dram_tensor("kw", [128, 8, _F], F32, kind="ExternalInput")
    ow_in = nc.dram_tensor("ow", [8, 2, _F], BF16, kind="ExternalInput")
    s1w_in = nc.dram_tensor("s1w", [128, 2], BF16, kind="ExternalInput")
    bc_in = nc.dram_tensor("bc", [128, 1], F32, kind="ExternalInput")
    y_out = nc.dram_tensor("y", [_BPC, 2, _F, _M], F32, kind="ExternalOutput")

    fast = (c_s == 0.0)

    with tile.TileContext(nc) as tc:
        with ExitStack() as ctx:
            wpool = ctx.enter_context(tc.tile_pool(name="w", bufs=1))
            xin = ctx.enter_context(tc.tile_pool(name="xin", bufs=3))
            xsqp = ctx.enter_context(tc.tile_pool(name="xsq", bufs=3))
            s1pool = ctx.enter_context(tc.tile_pool(name="s1", bufs=3))
            work = ctx.enter_context(tc.tile_pool(name="work", bufs=3))
            psd = ctx.enter_context(tc.tile_pool(name="psd", bufs=2, space="PSUM"))
            psw = ctx.enter_context(tc.tile_pool(name="psw", bufs=2, space="PSUM"))

            kw_t = wpool.tile([128, 8, _F], F32R)
            ow_t = wpool.tile([8, 2, _F], BF16)
            s1w_t = wpool.tile([128, 2], BF16)
            bc_t = wpool.tile([128, 1], F32)
            nc.sync.dma_start(out=kw_t, in_=kw_in[:, :, :].bitcast(F32R))
            nc.sync.dma_start(out=ow_t, in_=ow_in[:, :, :])
            nc.sync.dma_start(out=s1w_t, in_=s1w_in[:, :])
            nc.sync.dma_start(out=bc_t, in_=bc_in[:, :])

            for bi in range(_BPC):
                xeo = xin.tile([128, _M + _PAD], F32R)
                nc.sync.dma_start(
                    out=xeo,
                    in_=x_in[bi, :, :, :].flatten_outer_dims().bitcast(F32R),
                )
                xsq = xsqp.tile([128, _M + _PAD], BF16)
                nc.scalar.activation(
                    out=xsq, in_=xeo, func=mybir.ActivationFunctionType.Square
                )

                # s1[2, M+PAD]: per-parity channel sum of squares
                s1row = s1pool.tile([2, _M + _PAD], BF16)
                nc.vector.memset(s1row[:, _M : _M + _PAD], 0.0)
                for j in range(_M // _MT):
                    s1p = psw.tile([2, _MT], F32, tag="ws")
                    nc.tensor.matmul(
                        out=s1p,
                        lhsT=s1w_t[:, :],
                        rhs=xsq[:, j * _MT : (j + 1) * _MT],
                        start=True, stop=True,
                    )
                    if j % 2 == 0:
                        nc.scalar.copy(
                            out=s1row[:, j * _MT : (j + 1) * _MT], in_=s1p
                        )
                    else:
                        nc.vector.tensor_copy(
                            out=s1row[:, j * _MT : (j + 1) * _MT], in_=s1p
                        )

                # s1m[8, M+4]: row-pair j = s1row shifted by j columns.
                # Even-parity ws = rows 0..6; odd-parity ws = rows 1..7.
                s1m = s1pool.tile([8, _M + 4], BF16)
                for j in range(4):
                    nc.sync.dma_start(
                        out=s1m[2 * j : 2 * j + 2, :],
                        in_=s1row[:, j : j + _M + 4],
                    )

                for mt in range(_M // _MT):
                    m0 = mt * _MT
                    p_dot = psd.tile([128, 2, _MT], F32)
                    p_ws = psw.tile([128, 2, _MT], F32, tag="ws")
                    for pe in range(2):
                        for q in range(4):
                            nc.tensor.matmul(
                                out=p_dot[:, pe, :],
                                lhsT=kw_t[:, 4 * pe + q, :],
                                rhs=xeo[:, m0 + q : m0 + q + _MT],
                                start=(q == 0),
                                stop=(q == 3),
                            )
                        nc.tensor.matmul(
                            out=p_ws[:, pe, :],
                            lhsT=ow_t[:, pe, :],
                            rhs=s1m[:, m0 : m0 + _MT],
                            start=True, stop=True,
                        )

                    o_t = work.tile([128, 2, _MT], F32)
                    if fast:
                        r2 = work.tile([128, 2, _MT], F32)
                        nc.vector.reciprocal_approx_fast(
                            out=r2.bitcast(F32), in_=p_ws
                        )
                        dsq = work.tile([128, 2, _MT], F32)
                        nc.scalar.activation(
                            out=dsq, in_=p_dot,
                            func=mybir.ActivationFunctionType.Square,
                            scale=float(g_s),
                        )
                        t_t = work.tile([128, 2, _MT], F32)
                        nc.vector.tensor_tensor(
                            out=t_t, in0=dsq, in1=r2, op=mybir.AluOpType.mult
                        )
                        if mt % 2 == 0:
                            nc.scalar.activation(
                                out=o_t, in_=t_t,
                                func=mybir.ActivationFunctionType.Identity,
                                bias=bc_t[:, :], scale=1.0,
                            )
                        else:
                            nc.vector.tensor_scalar(
                                out=o_t, in0=t_t, scalar1=bc_t[:, :],
                                scalar2=None, op0=mybir.AluOpType.add,
                            )
                    else:
                        nrm = work.tile([128, 2, _MT], F32)
                        nc.scalar.activation(
                            out=nrm, in_=p_ws,
                            func=mybir.ActivationFunctionType.Sqrt,
                            bias=float(_EPS),
                        )
                        r_t = work.tile([128, 2, _MT], F32)
                        nc.vector.reciprocal_approx_fast(
                            out=r_t.bitcast(F32), in_=nrm
                        )
                        t_t = work.tile([128, 2, _MT], F32)
                        nc.vector.tensor_tensor(
                            out=t_t, in0=p_dot, in1=r_t, op=mybir.AluOpType.mult
                        )
                        q_t = work.tile([128, 2, _MT], F32)
                        nc.scalar.activation(
                            out=q_t, in_=t_t,
                            func=mybir.ActivationFunctionType.Square,
                            scale=float(g_s), bias=float(c_s),
                        )
                        nc.vector.tensor_scalar(
                            out=o_t, in0=q_t, scalar1=bc_t[:, :],
                            scalar2=None, op0=mybir.AluOpType.add,
                        )

                    for pe in range(2):
                        nc.sync.dma_start(
                            out=y_out[bi, pe, :, m0 : m0 + _MT],
                            in_=o_t[:, pe, :],
                        )
    nc.finalize()
    return nc


def _pack_inputs(x, k, b):
    xt = np.ascontiguousarray(x.transpose(0, 2, 1))        # [B, CIN, L]
    x_eo = np.zeros((_B, 2, _CIN, _M + _PAD), np.float32)
    x_eo[:, 0, :, :_M] = xt[:, :, 0::2]
    x_eo[:, 1, :, :_M] = xt[:, :, 1::2]

    kw = np.zeros((8, 128, _F), np.float32)
    # even parity: q0=k0|k1, q1=k2|k3, q2=k4|k5, q3=k6|0   (col offsets 0..3)
    kw[0, 0:64], kw[0, 64:128] = k[0], k[1]
    kw[1, 0:64], kw[1, 64:128] = k[2], k[3]
    kw[2, 0:64], kw[2, 64:128] = k[4], k[5]
    kw[3, 0:64] = k[6]
    # odd parity: q0=0|k0, q1=k1|k2, q2=k3|k4, q3=k5|k6    (col offsets 0..3)
    kw[4, 64:128] = k[0]
    kw[5, 0:64], kw[5, 64:128] = k[1], k[2]
    kw[6, 0:64], kw[6, 64:128] = k[3], k[4]
    kw[7, 0:64], kw[7, 64:128] = k[5], k[6]
    kw_dev = np.ascontiguousarray(kw.transpose(1, 0, 2))   # [128, 8, F]

    # ws lhsT over s1m rows: even = rows 0..6, odd = rows 1..7
    import ml_dtypes as _mld
    ow = np.zeros((8, 2, _F), _mld.bfloat16)
    ow[0:7, 0, :] = 1.0
    ow[1:8, 1, :] = 1.0

    # s1 lhsT [128, 2]: col 0 sums the even deck, col 1 the odd deck
    import ml_dtypes
    s1w = np.zeros((128, 2), ml_dtypes.bfloat16)
    s1w[0:64, 0] = 1.0
    s1w[64:128, 1] = 1.0

    bc = np.ascontiguousarray(b.reshape(_F, 1)).astype(np.float32)
    return x_eo, kw_dev, ow, s1w, bc


def kernel(x, k, b, g, c):
    x = np.asarray(x, dtype=np.float32)
    k = np.asarray(k, dtype=np.float32)
    b = np.asarray(b, dtype=np.float32)
    g_s = float(np.asarray(g).reshape(-1)[0])
    c_s = float(np.asarray(c).reshape(-1)[0])
    assert x.shape == (_B, _L, _CIN), x.shape
    assert k.shape == (_KS, _CIN, _F), k.shape

    key = (g_s, c_s)
    if key not in _prog_cache:
        _prog_cache[key] = _build_program(g_s, c_s)
    nc = _prog_cache[key]

    x_eo, kw_dev, ow, s1w, bc = _pack_inputs(x, k, b)
    in_maps = [
        {
            "x": np.ascontiguousarray(x_eo[i * _BPC : (i + 1) * _BPC]),
            "kw": kw_dev,
            "ow": ow,
            "s1w": s1w,
            "bc": bc,
        }
        for i in range(_NCORES)
    ]
    res = run_bass_kernel_spmd(nc, in_maps, list(range(_NCORES)))
    y_dev = np.concatenate([r["y"] for r in res.results], axis=0)  # [B,2,F,M]
    y = y_dev.transpose(0, 3, 1, 2).reshape(_B, _L, _F)[:, :_LP, :]
    return np.ascontiguousarray(y, dtype=np.float32)


# revision 6
# speedup vs baseline: 1.1063x; 1.1063x over previous
"""Trainium2 Bass kernel for BioNormalizedPolynomialCKN1D.

Computes, for x[B=64, L=4096, CIN=64], k[7, 64, 128], b[128], g, c (scalars):
    dot = conv_valid(x, k); ws = conv_valid(x*x, ones)       # [B, 4090, *]
    out = (g * dot / sqrt(ws + eps))**2 + b

Strategy (8 NeuronCores, data-parallel over batch, 8 batches/core):
  - Host packs x even/odd interleaved + channel-transposed:
      x_eo[b, p, ci, m] = x[b, 2m+p, ci]  -> SBUF tile XEO[128, M+PAD]
    with partitions = (parity*64 + ci). The 7-tap conv becomes 4
    accumulating K<=128 fp32r matmuls per output parity (tap pairs sit on
    the two 64-partition decks at one column offset) — no on-chip
    transposes or shifts.
  - Windowed sum-of-squares via a two-level scheme:
      s1[p, m] = sum_ci x^2  (one K=128 matmul, M=2)
      s1m[8, C]: row-pairs = s1 at col offsets 0..3 (4 tiny SBUF DMAs)
      ws broadcast [128, N] = ones.T @ s1m   (K=8 matmul)
    where the SAME s1m serves both parities: even taps = rows 0..6,
    odd taps = rows 1..7 (selected by zero rows in the ones lhsT).
  - Pointwise, spread across DVE/ACT (GPSIMD avoided entirely - it is
    ~17x slower than spec on tensor_scalar and its SBUF port is shared
    with DVE): r2 = reciprocal_approx_fast(ws) [DVE], dsq = Square(g*dot)
    [ACT], t = dsq*r2 [DVE], out = t + b [alternating ACT bias-add / DVE
    tensor_scalar for balance].
  - Device output layout [b, parity, F, m]; host inverse-permutes.

eps: ws ~ chi2(448) >= O(100) for this input distribution, so eps=1e-7
is relatively < 1e-9 and the fast path omits it. The general path (c!=0)
applies it exactly via the ACT Sqrt bias.
"""

import numpy as np
from contextlib import ExitStack

import concourse.tile as tile
from concourse import mybir, bacc
from concourse.bass_utils import run_bass_kernel_spmd

_B, _L, _CIN, _F, _KS = 64, 4096, 64, 128, 7
_LP = _L - _KS + 1           # 4090
_M = _L // 2                 # 2048 columns per parity
_PAD = 8
_MT = 512                    # matmul moving tile (one PSUM bank of fp32)
_NCORES = 8
_BPC = _B // _NCORES
_EPS = 1e-7

F32 = mybir.dt.float32
F32R = mybir.dt.float32r
BF16 = mybir.dt.bfloat16

_prog_cache = {}


def _build_program(g_s: float, c_s: float):
    nc = bacc.Bacc("TRN2", target_bir_lowering=False)
    x_in = nc.dram_tensor("x", [_BPC, 2, _CIN, _M + _PAD], F32, kind="ExternalInput")
    kw_in = nc.dram_tensor("kw", [128, 8, _F], F32, kind="ExternalInput")
    ow_in = nc.dram_tensor("ow", [8, 2, _F], BF16, kind="ExternalInput")
    s1w_in = nc.dram_tensor("s1w", [128, 2], BF16, kind="ExternalInput")
    bc_in = nc.dram_tensor("bc", [128, 1], F32, kind="ExternalInput")
    y_out = nc.dram_tensor("y", [_BPC, 2, _F, _M], F32, kind="ExternalOutput")

    fast = (c_s == 0.0)

    with tile.TileContext(nc) as tc:
        with ExitStack() as ctx:
            wpool = ctx.enter_context(tc.tile_pool(name="w", bufs=1))
            xin = ctx.enter_context(tc.tile_pool(name="xin", bufs=3))
            xsqp = ctx.enter_context(tc.tile_pool(name="xsq", bufs=3))
            s1pool = ctx.enter_context(tc.tile_pool(name="s1", bufs=3))
            work = ctx.enter_context(tc.tile_pool(name="work", bufs=3))
            psd = ctx.enter_context(tc.tile_pool(name="psd", bufs=2, space="PSUM"))
            psw = ctx.enter_context(tc.tile_pool(name="psw", bufs=1, space="PSUM"))
            pss = ctx.enter_context(tc.tile_pool(name="pss", bufs=2, space="PSUM"))

            kw_t = wpool.tile([128, 8, _F], F32R)
            ow_t = wpool.tile([8, 2, _F], BF16)
            s1w_t = wpool.tile([128, 2], BF16)
            bc_t = wpool.tile([128, 1], F32)
            nc.sync.dma_start(out=kw_t, in_=kw_in[:, :, :].bitcast(F32R))
            nc.sync.dma_start(out=ow_t, in_=ow_in[:, :, :])
            nc.sync.dma_start(out=s1w_t, in_=s1w_in[:, :])
            nc.sync.dma_start(out=bc_t, in_=bc_in[:, :])

            def emit_prologue(bi):
                xeo = xin.tile([128, _M + _PAD], F32R)
                nc.sync.dma_start(
                    out=xeo,
                    in_=x_in[bi, :, :, :].flatten_outer_dims().bitcast(F32R),
                )
                xsq = xsqp.tile([128, _M + _PAD], BF16)
                nc.scalar.activation(
                    out=xsq, in_=xeo, func=mybir.ActivationFunctionType.Square
                )
                # s1[2, M+PAD]: per-parity channel sum of squares
                s1row = s1pool.tile([2, _M + _PAD], BF16)
                nc.vector.memset(s1row[:, _M : _M + _PAD], 0.0)
                for j in range(_M // _MT):
                    s1p = pss.tile([2, _MT], F32)
                    nc.tensor.matmul(
                        out=s1p,
                        lhsT=s1w_t[:, :],
                        rhs=xsq[:, j * _MT : (j + 1) * _MT],
                        start=True, stop=True,
                    )
                    if j % 2 == 0:
                        nc.scalar.copy(
                            out=s1row[:, j * _MT : (j + 1) * _MT], in_=s1p
                        )
                    else:
                        nc.vector.tensor_copy(
                            out=s1row[:, j * _MT : (j + 1) * _MT], in_=s1p
                        )
                # s1m[8, M+4]: row-pair j = s1row shifted by j columns.
                # Even-parity ws = rows 0..6; odd-parity ws = rows 1..7.
                s1m = s1pool.tile([8, _M + 4], BF16)
                for j in range(4):
                    nc.sync.dma_start(
                        out=s1m[2 * j : 2 * j + 2, :],
                        in_=s1row[:, j : j + _M + 4],
                    )
                return xeo, s1m

            def emit_mtile(bi, xeo, s1m, mt):
                m0 = mt * _MT
                p_dot = psd.tile([128, 2, _MT], F32)
                p_ws = psw.tile([128, 2, _MT], F32)
                for pe in range(2):
                    for q in range(4):
                        nc.tensor.matmul(
                            out=p_dot[:, pe, :],
                            lhsT=kw_t[:, 4 * pe + q, :],
                            rhs=xeo[:, m0 + q : m0 + q + _MT],
                            start=(q == 0),
                            stop=(q == 3),
                        )
                    nc.tensor.matmul(
                        out=p_ws[:, pe, :],
                        lhsT=ow_t[:, pe, :],
                        rhs=s1m[:, m0 : m0 + _MT],
                        start=True, stop=True,
                    )

                o_t = work.tile([128, 2, _MT], F32)
                if fast:
                    r2 = work.tile([128, 2, _MT], F32)
                    nc.vector.reciprocal_approx_fast(
                        out=r2.bitcast(F32), in_=p_ws
                    )
                    dsq = work.tile([128, 2, _MT], F32)
                    nc.scalar.activation(
                        out=dsq, in_=p_dot,
                        func=mybir.ActivationFunctionType.Square,
                        scale=float(g_s),
                    )
                    t_t = work.tile([128, 2, _MT], F32)
                    nc.vector.tensor_tensor(
                        out=t_t, in0=dsq, in1=r2, op=mybir.AluOpType.mult
                    )
                    if mt % 2 == 0:
                        nc.scalar.activation(
                            out=o_t, in_=t_t,
                            func=mybir.ActivationFunctionType.Identity,
                            bias=bc_t[:, :], scale=1.0,
                        )
                    else:
                        nc.vector.tensor_scalar(
                            out=o_t, in0=t_t, scalar1=bc_t[:, :],
                            scalar2=None, op0=mybir.AluOpType.add,
                        )
                else:
                    nrm = work.tile([128, 2, _MT], F32)
                    nc.scalar.activation(
                        out=nrm, in_=p_ws,
                        func=mybir.ActivationFunctionType.Sqrt,
                        bias=float(_EPS),
                    )
                    r_t = work.tile([128, 2, _MT], F32)
                    nc.vector.reciprocal_approx_fast(
                        out=r_t.bitcast(F32), in_=nrm
                    )
                    t_t = work.tile([128, 2, _MT], F32)
                    nc.vector.tensor_tensor(
                        out=t_t, in0=p_dot, in1=r_t, op=mybir.AluOpType.mult
                    )
                    q_t = work.tile([128, 2, _MT], F32)
                    nc.scalar.activation(
                        out=q_t, in_=t_t,
                        func=mybir.ActivationFunctionType.Square,
                        scale=float(g_s), bias=float(c_s),
                    )
                    nc.vector.tensor_scalar(
                        out=o_t, in0=q_t, scalar1=bc_t[:, :],
                        scalar2=None, op0=mybir.AluOpType.add,
                    )

                for pe in range(2):
                    nc.sync.dma_start(
                        out=y_out[bi, pe, :, m0 : m0 + _MT],
                        in_=o_t[:, pe, :],
                    )

            cur = emit_prologue(0)
            nxt = None
            for bi in range(_BPC):
                for mt in range(_M // _MT):
                    emit_mtile(bi, cur[0], cur[1], mt)
                    if mt == 0 and bi + 1 < _BPC:
                        nxt = emit_prologue(bi + 1)
                cur = nxt
    nc.finalize()
    return nc


def _pack_inputs(x, k, b):
    xt = np.ascontiguousarray(x.transpose(0, 2, 1))        # [B, CIN, L]
    x_eo = np.zeros((_B, 2, _CIN, _M + _PAD), np.float32)
    x_eo[:, 0, :, :_M] = xt[:, :, 0::2]
    x_eo[:, 1, :, :_M] = xt[:, :, 1::2]

    kw = np.zeros((8, 128, _F), np.float32)
    # even parity: q0=k0|k1, q1=k2|k3, q2=k4|k5, q3=k6|0   (col offsets 0..3)
    kw[0, 0:64], kw[0, 64:128] = k[0], k[1]
    kw[1, 0:64], kw[1, 64:128] = k[2], k[3]
    kw[2, 0:64], kw[2, 64:128] = k[4], k[5]
    kw[3, 0:64] = k[6]
    # odd parity: q0=0|k0, q1=k1|k2, q2=k3|k4, q3=k5|k6    (col offsets 0..3)
    kw[4, 64:128] = k[0]
    kw[5, 0:64], kw[5, 64:128] = k[1], k[2]
    kw[6, 0:64], kw[6, 64:128] = k[3], k[4]
    kw[7, 0:64], kw[7, 64:128] = k[5], k[6]
    kw_dev = np.ascontiguousarray(kw.transpose(1, 0, 2))   # [128, 8, F]

    # ws lhsT over s1m rows: even = rows 0..6, odd = rows 1..7
    import ml_dtypes as _mld
    ow = np.zeros((8, 2, _F), _mld.bfloat16)
    ow[0:7, 0, :] = 1.0
    ow[1:8, 1, :] = 1.0

    # s1 lhsT [128, 2]: col 0 sums the even deck, col 1 the odd deck
    import ml_dtypes
    s1w = np.zeros((128, 2), ml_dtypes.bfloat16)
    s1w[0:64, 0] = 1.0
    s1w[64:128, 1] = 1.0

    bc = np.ascontiguousarray(b.reshape(_F, 1)).astype(np.float32)
    return x_eo, kw_dev, ow, s1w, bc


def kernel(x, k, b, g, c):
    x = np.asarray(x, dtype=np.float32)
    k = np.asarray(k, dtype=np.float32)
    b = np.asarray(b, dtype=np.float32)
    g_s = float(np.asarray(g).reshape(-1)[0])
    c_s = float(np.asarray(c).reshape(-1)[0])
    assert x.shape == (_B, _L, _CIN), x.shape
    assert k.shape == (_KS, _CIN, _F), k.shape

    key = (g_s, c_s)
    if key not in _prog_cache:
        _prog_cache[key] = _build_program(g_s, c_s)
    nc = _prog_cache[key]

    x_eo, kw_dev, ow, s1w, bc = _pack_inputs(x, k, b)
    in_maps = [
        {
            "x": np.ascontiguousarray(x_eo[i * _BPC : (i + 1) * _BPC]),
            "kw": kw_dev,
            "ow": ow,
            "s1w": s1w,
            "bc": bc,
        }
        for i in range(_NCORES)
    ]
    res = run_bass_kernel_spmd(nc, in_maps, list(range(_NCORES)))
    y_dev = np.concatenate([r["y"] for r in res.results], axis=0)  # [B,2,F,M]
    y = y_dev.transpose(0, 3, 1, 2).reshape(_B, _L, _F)[:, :_LP, :]
    return np.ascontiguousarray(y, dtype=np.float32)


# revision 7
# speedup vs baseline: 1.2479x; 1.1280x over previous
"""Trainium2 Bass kernel for BioNormalizedPolynomialCKN1D.

Computes, for x[B=64, L=4096, CIN=64], k[7, 64, 128], b[128], g, c (scalars):
    dot = conv_valid(x, k); ws = conv_valid(x*x, ones)       # [B, 4090, *]
    out = (g * dot / sqrt(ws + eps))**2 + b

Strategy (8 NeuronCores, data-parallel over batch, 8 batches/core):
  - Host packs x even/odd interleaved + channel-transposed:
      x_eo[b, p, ci, m] = x[b, 2m+p, ci]  -> SBUF tile XEO[128, M+PAD]
    with partitions = (parity*64 + ci). The 7-tap conv becomes 4
    accumulating K<=128 fp32r matmuls per output parity (tap pairs sit on
    the two 64-partition decks at one column offset) — no on-chip
    transposes or shifts.
  - Windowed sum-of-squares via a two-level scheme:
      s1[p, m] = sum_ci x^2  (one K=128 matmul, M=2)
      s1m[8, C]: row-pairs = s1 at col offsets 0..3 (4 tiny SBUF DMAs)
      ws broadcast [128, N] = ones.T @ s1m   (K=8 matmul)
    where the SAME s1m serves both parities: even taps = rows 0..6,
    odd taps = rows 1..7 (selected by zero rows in the ones lhsT).
  - Pointwise, spread across DVE/ACT (GPSIMD avoided entirely - it is
    ~17x slower than spec on tensor_scalar and its SBUF port is shared
    with DVE): r2 = reciprocal_approx_fast(ws) [DVE], dsq = Square(g*dot)
    [ACT], t = dsq*r2 [DVE], out = t + b [alternating ACT bias-add / DVE
    tensor_scalar for balance].
  - Device output layout [b, parity, F, m]; host inverse-permutes.

eps: ws ~ chi2(448) >= O(100) for this input distribution, so eps=1e-7
is relatively < 1e-9 and the fast path omits it. The general path (c!=0)
applies it exactly via the ACT Sqrt bias.
"""

import numpy as np
from contextlib import ExitStack

import concourse.tile as tile
from concourse import mybir, bacc
from concourse.bass_utils import run_bass_kernel_spmd

_B, _L, _CIN, _F, _KS = 64, 4096, 64, 128, 7
_LP = _L - _KS + 1           # 4090
_M = _L // 2                 # 2048 columns per parity
_PAD = 8
_MT = 512                    # matmul moving tile (one PSUM bank of fp32)
_NCORES = 8
_BPC = _B // _NCORES
_EPS = 1e-7

F32 = mybir.dt.float32
F32R = mybir.dt.float32r
BF16 = mybir.dt.bfloat16

_prog_cache = {}


def _build_program(g_s: float, c_s: float):
    nc = bacc.Bacc("TRN2", target_bir_lowering=False)
    x_in = nc.dram_tensor("x", [_BPC, 2, _CIN, _M + _PAD], BF16, kind="ExternalInput")
    kw_in = nc.dram_tensor("kw", [128, 8, _F], BF16, kind="ExternalInput")
    ow_in = nc.dram_tensor("ow", [8, 2, _F], BF16, kind="ExternalInput")
    s1w_in = nc.dram_tensor("s1w", [128, 2], BF16, kind="ExternalInput")
    bc_in = nc.dram_tensor("bc", [128, 1], F32, kind="ExternalInput")
    y_out = nc.dram_tensor("y", [_BPC, 2, _F, _M], F32, kind="ExternalOutput")

    fast = (c_s == 0.0)

    with tile.TileContext(nc) as tc:
        with ExitStack() as ctx:
            wpool = ctx.enter_context(tc.tile_pool(name="w", bufs=1))
            xin = ctx.enter_context(tc.tile_pool(name="xin", bufs=3))
            xsqp = ctx.enter_context(tc.tile_pool(name="xsq", bufs=3))
            s1pool = ctx.enter_context(tc.tile_pool(name="s1", bufs=3))
            work = ctx.enter_context(tc.tile_pool(name="work", bufs=3))
            psd = ctx.enter_context(tc.tile_pool(name="psd", bufs=2, space="PSUM"))
            psw = ctx.enter_context(tc.tile_pool(name="psw", bufs=1, space="PSUM"))
            pss = ctx.enter_context(tc.tile_pool(name="pss", bufs=2, space="PSUM"))

            kw_t = wpool.tile([128, 8, _F], BF16)
            ow_t = wpool.tile([8, 2, _F], BF16)
            s1w_t = wpool.tile([128, 2], BF16)
            bc_t = wpool.tile([128, 1], F32)
            nc.sync.dma_start(out=kw_t, in_=kw_in[:, :, :])
            nc.sync.dma_start(out=ow_t, in_=ow_in[:, :, :])
            nc.sync.dma_start(out=s1w_t, in_=s1w_in[:, :])
            nc.sync.dma_start(out=bc_t, in_=bc_in[:, :])

            def emit_prologue(bi):
                xeo = xin.tile([128, _M + _PAD], BF16)
                nc.sync.dma_start(
                    out=xeo,
                    in_=x_in[bi, :, :, :].flatten_outer_dims(),
                )
                xsq = xsqp.tile([128, _M + _PAD], BF16)
                nc.scalar.activation(
                    out=xsq, in_=xeo, func=mybir.ActivationFunctionType.Square
                )
                # s1[2, M+PAD]: per-parity channel sum of squares
                s1row = s1pool.tile([2, _M + _PAD], BF16)
                nc.vector.memset(s1row[:, _M : _M + _PAD], 0.0)
                for j in range(_M // _MT):
                    s1p = pss.tile([2, _MT], F32)
                    nc.tensor.matmul(
                        out=s1p,
                        lhsT=s1w_t[:, :],
                        rhs=xsq[:, j * _MT : (j + 1) * _MT],
                        start=True, stop=True,
                    )
                    if j % 2 == 0:
                        nc.scalar.copy(
                            out=s1row[:, j * _MT : (j + 1) * _MT], in_=s1p
                        )
                    else:
                        nc.vector.tensor_copy(
                            out=s1row[:, j * _MT : (j + 1) * _MT], in_=s1p
                        )
                # s1m[8, M+4]: row-pair j = s1row shifted by j columns.
                # Even-parity ws = rows 0..6; odd-parity ws = rows 1..7.
                s1m = s1pool.tile([8, _M + 4], BF16)
                for j in range(4):
                    nc.sync.dma_start(
                        out=s1m[2 * j : 2 * j + 2, :],
                        in_=s1row[:, j : j + _M + 4],
                    )
                return xeo, s1m

            def emit_mtile(bi, xeo, s1m, mt):
                m0 = mt * _MT
                p_dot = psd.tile([128, 2, _MT], F32)
                p_ws = psw.tile([128, 2, _MT], F32)
                for pe in range(2):
                    for q in range(4):
                        nc.tensor.matmul(
                            out=p_dot[:, pe, :],
                            lhsT=kw_t[:, 4 * pe + q, :],
                            rhs=xeo[:, m0 + q : m0 + q + _MT],
                            start=(q == 0),
                            stop=(q == 3),
                        )
                    nc.tensor.matmul(
                        out=p_ws[:, pe, :],
                        lhsT=ow_t[:, pe, :],
                        rhs=s1m[:, m0 : m0 + _MT],
                        start=True, stop=True,
                    )

                o_t = work.tile([128, 2, _MT], F32)
                if fast:
                    r2 = work.tile([128, 2, _MT], F32)
                    nc.vector.reciprocal_approx_fast(
                        out=r2.bitcast(F32), in_=p_ws
                    )
                    dsq = work.tile([128, 2, _MT], F32)
                    nc.scalar.activation(
                        out=dsq, in_=p_dot,
                        func=mybir.ActivationFunctionType.Square,
                        scale=float(g_s),
                    )
                    t_t = work.tile([128, 2, _MT], F32)
                    nc.vector.tensor_tensor(
                        out=t_t, in0=dsq, in1=r2, op=mybir.AluOpType.mult
                    )
                    if mt % 2 == 0:
                        nc.scalar.activation(
                            out=o_t, in_=t_t,
                            func=mybir.ActivationFunctionType.Identity,
                            bias=bc_t[:, :], scale=1.0,
                        )
                    else:
                        nc.vector.tensor_scalar(
                            out=o_t, in0=t_t, scalar1=bc_t[:, :],
                            scalar2=None, op0=mybir.AluOpType.add,
                        )
                else:
                    nrm = work.tile([128, 2, _MT], F32)
                    nc.scalar.activation(
                        out=nrm, in_=p_ws,
                        func=mybir.ActivationFunctionType.Sqrt,
                        bias=float(_EPS),
                    )
                    r_t = work.tile([128, 2, _MT], F32)
                    nc.vector.reciprocal_approx_fast(
                        out=r_t.bitcast(F32), in_=nrm
                    )
                    t_t = work.tile([128, 2, _MT], F32)
                    nc.vector.tensor_tensor(
                        out=t_t, in0=p_dot, in1=r_t, op=mybir.AluOpType.mult
                    )
                    q_t = work.tile([128, 2, _MT], F32)
                    nc.scalar.activation(
                        out=q_t, in_=t_t,
                        func=mybir.ActivationFunctionType.Square,
                        scale=float(g_s), bias=float(c_s),
                    )
                    nc.vector.tensor_scalar(
                        out=o_t, in0=q_t, scalar1=bc_t[:, :],
                        scalar2=None, op0=mybir.AluOpType.add,
                    )

                for pe in range(2):
                    nc.sync.dma_start(
                        out=y_out[bi, pe, :, m0 : m0 + _MT],
                        in_=o_t[:, pe, :],
                    )

            cur = emit_prologue(0)
            nxt = None
            for bi in range(_BPC):
                for mt in range(_M // _MT):
                    emit_mtile(bi, cur[0], cur[1], mt)
                    if mt == 0 and bi + 1 < _BPC:
                        nxt = emit_prologue(bi + 1)
                cur = nxt
    nc.finalize()
    return nc


def _pack_inputs(x, k, b):
    import ml_dtypes
    xt = np.ascontiguousarray(x.transpose(0, 2, 1))        # [B, CIN, L]
    x_eo = np.zeros((_B, 2, _CIN, _M + _PAD), ml_dtypes.bfloat16)
    x_eo[:, 0, :, :_M] = xt[:, :, 0::2]
    x_eo[:, 1, :, :_M] = xt[:, :, 1::2]

    kw = np.zeros((8, 128, _F), ml_dtypes.bfloat16)
    # even parity: q0=k0|k1, q1=k2|k3, q2=k4|k5, q3=k6|0   (col offsets 0..3)
    kw[0, 0:64], kw[0, 64:128] = k[0], k[1]
    kw[1, 0:64], kw[1, 64:128] = k[2], k[3]
    kw[2, 0:64], kw[2, 64:128] = k[4], k[5]
    kw[3, 0:64] = k[6]
    # odd parity: q0=0|k0, q1=k1|k2, q2=k3|k4, q3=k5|k6    (col offsets 0..3)
    kw[4, 64:128] = k[0]
    kw[5, 0:64], kw[5, 64:128] = k[1], k[2]
    kw[6, 0:64], kw[6, 64:128] = k[3], k[4]
    kw[7, 0:64], kw[7, 64:128] = k[5], k[6]
    kw_dev = np.ascontiguousarray(kw.transpose(1, 0, 2))   # [128, 8, F]

    # ws lhsT over s1m rows: even = rows 0..6, odd = rows 1..7
    import ml_dtypes as _mld
    ow = np.zeros((8, 2, _F), _mld.bfloat16)
    ow[0:7, 0, :] = 1.0
    ow[1:8, 1, :] = 1.0

    # s1 lhsT [128, 2]: col 0 sums the even deck, col 1 the odd deck
    import ml_dtypes
    s1w = np.zeros((128, 2), ml_dtypes.bfloat16)
    s1w[0:64, 0] = 1.0
    s1w[64:128, 1] = 1.0

    bc = np.ascontiguousarray(b.reshape(_F, 1)).astype(np.float32)
    return x_eo, kw_dev, ow, s1w, bc


def kernel(x, k, b, g, c):
    x = np.asarray(x, dtype=np.float32)
    k = np.asarray(k, dtype=np.float32)
    b = np.asarray(b, dtype=np.float32)
    g_s = float(np.asarray(g).reshape(-1)[0])
    c_s = float(np.asarray(c).reshape(-1)[0])
    assert x.shape == (_B, _L, _CIN), x.shape
    assert k.shape == (_KS, _CIN, _F), k.shape

    key = (g_s, c_s)
    if key not in _prog_cache:
        _prog_cache[key] = _build_program(g_s, c_s)
    nc = _prog_cache[key]

    x_eo, kw_dev, ow, s1w, bc = _pack_inputs(x, k, b)
    in_maps = [
        {
            "x": np.ascontiguousarray(x_eo[i * _BPC : (i + 1) * _BPC]),
            "kw": kw_dev,
            "ow": ow,
            "s1w": s1w,
            "bc": bc,
        }
        for i in range(_NCORES)
    ]
    res = run_bass_kernel_spmd(nc, in_maps, list(range(_NCORES)))
    y_dev = np.concatenate([r["y"] for r in res.results], axis=0)  # [B,2,F,M]
    y = y_dev.transpose(0, 3, 1, 2).reshape(_B, _L, _F)[:, :_LP, :]
    return np.ascontiguousarray(y, dtype=np.float32)
